# revision 3
# baseline (speedup 1.0000x reference)
"""Trainium2 Bass kernel for nn_AttentionHeteroRGCNLayer.

Math: softmax of a length-1 vector is 1.0, so the per-relation attention
weights are w = softmax([1,1,1]) = 1/3 each (computed generally anyway).
h = feat @ Wc with Wc = sum_r w_r W_r, and aggregation is linear, so
out = LN(relu( (sum_r D_r^-1 A_r (feat @ Wc)) / 1 )) with per-edge weight
w_e = w_r / max(deg_r[dst_e], 1) folded into the one-hot scatter matrix:
    agg_feat[dst] = sum_e w_e * feat[src_e]     (edge-parallel, dst-sharded)
    out[dst]      = LN(relu(agg_feat[dst] @ Wc))

Distribution: dst-range sharding across 8 cores (6400 dst rows each, N padded
to 51200); feat table replicated to every core (no collectives). Edges are
bucketed by 128-dst windows on the host; the device gathers source rows with
dma_gather (bf16, ~2176 rows/call across 4 SWDGE queues), builds one-hot
scatter matrices with broadcast is_equal ops, and aggregates with bf16
TensorEngine matmuls into PSUM. Per window, Wc is applied via two PE
transposes + two matmuls, then ReLU + LayerNorm fused on ACT/DVE.
"""
import os
import numpy as np
import ml_dtypes

import concourse.bacc as bacc
import concourse.bass as bass
import concourse.mybir as mybir
import concourse.tile as tile
from concourse.bass_utils import run_bass_kernel_spmd

BF16 = mybir.dt.bfloat16
F32 = mybir.dt.float32
NP_BF16 = np.dtype(ml_dtypes.bfloat16)

N = 50000
D = 256
P = 128
NC = 8
NPAD = 51200            # 8 * 6400
ROWS_PER_CORE = NPAD // NC      # 6400
WINS_PER_CORE = ROWS_PER_CORE // P   # 50
SUPER = 2               # windows per superwindow
SUPERS_PER_CORE = WINS_PER_CORE // SUPER  # 25
LO_SPLIT = 32768        # int16 index limit
MAX_TILES_PER_CALL = 17  # ~2176 indices per dma_gather call
LN_EPS = 1e-5
NQ = 4                  # SWDGE queues


def _bf16(x):
    return np.asarray(x, dtype=np.float32).astype(NP_BF16)


def _softmax(v):
    e = np.exp(v - v.max())
    return e / e.sum()


def _host_prep(feat, W0, W1, W2, a0, a1, a2, srcs, dsts):
    """Sort/bucket edges, build per-core gather metadata."""
    w3 = _softmax(np.concatenate([_softmax(np.asarray(a, np.float64).ravel())
                                  for a in (a0, a1, a2)]))
    Wc = (w3[0] * W0 + w3[1] * W1 + w3[2] * W2).astype(np.float32)

    src_all, dst_all, wgt_all = [], [], []
    for r in range(3):
        s = np.asarray(srcs[r], np.int64)
        d = np.asarray(dsts[r], np.int64)
        deg = np.bincount(d, minlength=N)
        w_e = (w3[r] / np.maximum(deg, 1.0)[d]).astype(np.float32)
        src_all.append(s); dst_all.append(d); wgt_all.append(w_e)
    src_all = np.concatenate(src_all)
    dst_all = np.concatenate(dst_all)
    wgt_all = np.concatenate(wgt_all)

    order = np.argsort(dst_all, kind="stable")
    s_s, d_s, w_s = src_all[order], dst_all[order], wgt_all[order]

    # global window = dst // 128 (0..399); core = gw // 50
    gw = (d_s // P).astype(np.int64)
    win_counts = np.bincount(gw, minlength=NC * WINS_PER_CORE)
    win_start = np.zeros(NC * WINS_PER_CORE + 1, np.int64)
    np.cumsum(win_counts, out=win_start[1:])

    # per (core, window) lo/hi edge lists
    lo_edges = {}
    hi_edges = {}
    n_lo = np.zeros((NC, WINS_PER_CORE), np.int64)
    n_hi = np.zeros((NC, WINS_PER_CORE), np.int64)
    for g in range(NC * WINS_PER_CORE):
        c, w = g // WINS_PER_CORE, g % WINS_PER_CORE
        a, b = win_start[g], win_start[g + 1]
        sl_s, sl_d, sl_w = s_s[a:b], d_s[a:b], w_s[a:b]
        m = sl_s < LO_SPLIT
        lo_edges[(c, w)] = (sl_s[m], sl_d[m], sl_w[m])
        hi_edges[(c, w)] = (sl_s[~m] - LO_SPLIT, sl_d[~m], sl_w[~m])
        n_lo[c, w] = int(m.sum())
        n_hi[c, w] = int((~m).sum())

    # cross-core-uniform tile counts per window
    T_lo = np.maximum(1, -(-n_lo.max(axis=0) // P))   # [50]
    T_hi = np.maximum(1, -(-n_hi.max(axis=0) // P))   # [50]

    # superwindow layout: tiles ordered [loA, loB, hiA, hiB]
    # build schedule (shared across cores)
    schedule = []           # per super: dict with tile layout + calls
    total_tiles = 0
    for s in range(SUPERS_PER_CORE):
        wA, wB = SUPER * s, SUPER * s + 1
        tLA, tLB = int(T_lo[wA]), int(T_lo[wB])
        tHA, tHB = int(T_hi[wA]), int(T_hi[wB])
        tl, th = tLA + tLB, tHA + tHB
        calls = []
        # lo calls (into xlo), tile-granular, <= MAX_TILES_PER_CALL each
        ncall_lo = -(-tl // MAX_TILES_PER_CALL)
        ofs = 0
        for j in range(ncall_lo):
            cnt = (tl - ofs + (ncall_lo - j) - 1) // (ncall_lo - j)
            calls.append(("lo", ofs, cnt))
            ofs += cnt
        ncall_hi = -(-th // MAX_TILES_PER_CALL)
        ofs = 0
        for j in range(ncall_hi):
            cnt = (th - ofs + (ncall_hi - j) - 1) // (ncall_hi - j)
            calls.append(("hi", ofs, cnt))
            ofs += cnt
        schedule.append(dict(
            tile_base=total_tiles, tLA=tLA, tLB=tLB, tHA=tHA, tHB=tHB,
            tl=tl, th=th, calls=calls,
        ))
        total_tiles += tl + th

    # host metadata arrays per core
    idx16 = np.zeros((NC, P, total_tiles * (P // 16)), np.int16)
    dlmat = np.full((NC, P, total_tiles), -1.0, np.float32)
    wgmat = np.zeros((NC, P, total_tiles), np.float32)

    for c in range(NC):
        for s in range(SUPERS_PER_CORE):
            sc = schedule[s]
            wA, wB = SUPER * s, SUPER * s + 1
            base = sc["tile_base"]
            # tile-block layout: [loA(tLA), loB(tLB), hiA(tHA), hiB(tHB)]
            blocks = [
                (lo_edges[(c, wA)], wA, base, sc["tLA"]),
                (lo_edges[(c, wB)], wB, base + sc["tLA"], sc["tLB"]),
                (hi_edges[(c, wA)], wA, base + sc["tl"], sc["tHA"]),
                (hi_edges[(c, wB)], wB, base + sc["tl"] + sc["tHA"], sc["tHB"]),
            ]
            for (es, ed, ew), w, tbase, tcnt in blocks:
                cap = tcnt * P
                n = len(es)
                assert n <= cap
                sidx = np.zeros(cap, np.int64)
                sidx[:n] = es
                dloc = np.full(cap, -1.0, np.float32)
                dloc[:n] = (ed - (c * ROWS_PER_CORE + w * P)).astype(np.float32)
                wv = np.zeros(cap, np.float32)
                wv[:n] = ew
                # slot q -> tile tbase + q//128, partition q%128
                idx_wrap = sidx.reshape(tcnt, P)          # [t, p] p = q%128? no:
                # within one gather call position i -> [i%128 partition, i//128 tile]
                # our slots q are call-relative only per call; but calls cover
                # tile ranges contiguously, so q within block == call pos up to
                # tile offset, and i%128/i//128 mapping holds per 128-chunk.
                dlmat[c, :, tbase:tbase + tcnt] = dloc.reshape(tcnt, P).T
                wgmat[c, :, tbase:tbase + tcnt] = wv.reshape(tcnt, P).T
                # idx: per tile, wrapped [16, 8]: index i of tile -> [i%16, i//16]
                iw = idx_wrap.reshape(tcnt, 8, 16).transpose(0, 2, 1)  # [t,16,8]
                iw = np.tile(iw, (1, 8, 1))                            # [t,128,8]
                idx16[c, :, tbase * 8:(tbase + tcnt) * 8] = (
                    iw.transpose(1, 0, 2).reshape(P, tcnt * 8).astype(np.int16))

    return dict(
        Wc=Wc, schedule=schedule, total_tiles=total_tiles,
        idx16=idx16, dlmat=dlmat, wgmat=wgmat,
    )


def _build_nc(schedule, total_tiles, apply_affine):
    nc = bacc.Bacc(None, target_bir_lowering=False, num_swdge_queues=NQ)
    tab_lo = nc.declare_dram_parameter("tab_lo", [LO_SPLIT, D], BF16, isOutput=False)
    tab_hi = nc.declare_dram_parameter("tab_hi", [N - LO_SPLIT, D], BF16, isOutput=False)
    idx_d = nc.declare_dram_parameter("idx", [P, total_tiles * 8], mybir.dt.int16, isOutput=False)
    dl_d = nc.declare_dram_parameter("dl", [P, total_tiles], BF16, isOutput=False)
    wg_d = nc.declare_dram_parameter("wg", [P, total_tiles], BF16, isOutput=False)
    wc_d = nc.declare_dram_parameter("wc", [P, 2 * D], BF16, isOutput=False)
    cst_d = nc.declare_dram_parameter("cst", [P, 2 * P], BF16, isOutput=False)  # iota | identity
    gb_d = nc.declare_dram_parameter("gb", [P, 2 * D], F32, isOutput=False)     # gamma | beta
    out_d = nc.declare_dram_parameter("out", [ROWS_PER_CORE, D], F32, isOutput=True)

    max_tl = max(sc["tl"] for sc in schedule)
    max_th = max(sc["th"] for sc in schedule)
    max_tot = max(sc["tl"] + sc["th"] for sc in schedule)

    qrot = [0]

    with tile.TileContext(nc) as tc:
        with (
            tc.tile_pool(name="meta", bufs=1) as meta_pool,
            tc.tile_pool(name="xlo", bufs=2) as xlo_pool,
            tc.tile_pool(name="xhi", bufs=2) as xhi_pool,
            tc.tile_pool(name="bmat", bufs=2) as b_pool,
            tc.tile_pool(name="ev", bufs=3) as ev_pool,
            tc.tile_pool(name="st", bufs=4) as st_pool,
            tc.tile_pool(name="psA", bufs=2, space="PSUM") as psA,
            tc.tile_pool(name="psB", bufs=2, space="PSUM") as psB,
            tc.tile_pool(name="psC", bufs=2, space="PSUM") as psC,
        ):
            idx_sb = meta_pool.tile([P, total_tiles * 8], mybir.dt.int16)
            nc.sync.dma_start(out=idx_sb[:], in_=idx_d[:])
            mrow = meta_pool.tile([P, 2 * total_tiles + 2 * D + 2 * P], BF16)
            nc.sync.dma_start(out=mrow[:, :total_tiles], in_=dl_d[:])
            nc.sync.dma_start(out=mrow[:, total_tiles:2 * total_tiles], in_=wg_d[:])
            nc.sync.dma_start(out=mrow[:, 2 * total_tiles:2 * total_tiles + 2 * D], in_=wc_d[:])
            nc.sync.dma_start(out=mrow[:, 2 * total_tiles + 2 * D:], in_=cst_d[:])
            dl_sb = mrow[:, 0:total_tiles]
            wg_sb = mrow[:, total_tiles:2 * total_tiles]
            wc_sb = mrow[:, 2 * total_tiles:2 * total_tiles + 2 * D]
            iota_sb = mrow[:, 2 * total_tiles + 2 * D:2 * total_tiles + 2 * D + P]
            ident_sb = mrow[:, 2 * total_tiles + 2 * D + P:]
            gb_sb = meta_pool.tile([P, 2 * D], F32)
            nc.sync.dma_start(out=gb_sb[:], in_=gb_d[:])
            gamma_sb = gb_sb[:, 0:D]
            beta_sb = gb_sb[:, D:2 * D]

            for s in range(SUPERS_PER_CORE):
                sc = schedule[s]
                base = sc["tile_base"]
                tl, th = sc["tl"], sc["th"]
                xlo = xlo_pool.tile([P, max_tl * D], BF16, tag="xlo")
                xhi = xhi_pool.tile([P, max_th * D], BF16, tag="xhi")
                for kind, ofs, cnt in sc["calls"]:
                    x_t, tab, tofs = (xlo, tab_lo, base + ofs) if kind == "lo" \
                        else (xhi, tab_hi, base + tl + ofs)
                    ni = cnt * P
                    nc.gpsimd.dma_gather(
                        out_ap=x_t[:, ofs * D:(ofs + cnt) * D].rearrange(
                            "p (t e) -> p t e", e=D),
                        in_ap=tab[:],
                        idxs_ap=idx_sb[:, tofs * 8:(tofs + cnt) * 8],
                        num_idxs=ni,
                        num_idxs_reg=ni,
                        elem_size=D,
                        single_packet=False,
                        queue_num=qrot[0] % NQ,
                    )
                    qrot[0] += 1

                ntile = tl + th
                bmat = b_pool.tile([P, max_tot * P], BF16, tag="b")
                nc.vector.tensor_tensor(
                    out=bmat[:, :ntile * P].rearrange("p (t c) -> p t c", c=P),
                    in0=iota_sb.unsqueeze(1).to_broadcast([P, ntile, P]),
                    in1=dl_sb[:, base:base + ntile].unsqueeze(2).to_broadcast([P, ntile, P]),
                    op=mybir.AluOpType.is_equal,
                )
                nc.vector.tensor_tensor(
                    out=bmat[:, :ntile * P].rearrange("p (t c) -> p t c", c=P),
                    in0=bmat[:, :ntile * P].rearrange("p (t c) -> p t c", c=P),
                    in1=wg_sb[:, base:base + ntile].unsqueeze(2).to_broadcast([P, ntile, P]),
                    op=mybir.AluOpType.mult,
                )

                # per-window aggregation + fused Wc/relu/LN/store
                for wi in range(SUPER):
                    w = SUPER * s + wi
                    # tile ids (within super) for this window: lo block + hi block
                    if wi == 0:
                        lo_t = list(range(0, sc["tLA"]))
                        hi_t = list(range(tl, tl + sc["tHA"]))
                    else:
                        lo_t = list(range(sc["tLA"], tl))
                        hi_t = list(range(tl + sc["tHA"], tl + th))
                    tiles = lo_t + hi_t
                    agg = psA.tile([P, D], F32, tag="agg")
                    for k, t in enumerate(tiles):
                        xsrc = xlo if t < tl else xhi
                        xoff = t if t < tl else t - tl
                        nc.tensor.matmul(
                            out=agg[:],
                            lhsT=bmat[:, t * P:(t + 1) * P],
                            rhs=xsrc[:, xoff * D:(xoff + 1) * D],
                            start=(k == 0), stop=(k == len(tiles) - 1),
                        )
                    aggS = ev_pool.tile([P, D], BF16, tag="aggS")
                    nc.scalar.activation(out=aggS[:], in_=agg[:],
                                         func=mybir.ActivationFunctionType.Copy)
                    trp = psB.tile([P, D], BF16, tag="trp")
                    nc.tensor.transpose(out=trp[:, 0:P], in_=aggS[:, 0:P], identity=ident_sb)
                    nc.tensor.transpose(out=trp[:, P:D], in_=aggS[:, P:D], identity=ident_sb)
                    aggT = ev_pool.tile([P, D], BF16, tag="aggT")
                    nc.scalar.activation(out=aggT[:], in_=trp[:],
                                         func=mybir.ActivationFunctionType.Copy)
                    out2 = psC.tile([P, D], F32, tag="out2")
                    nc.tensor.matmul(out=out2[:], lhsT=aggT[:, 0:P],
                                     rhs=wc_sb[:, 0:D], start=True, stop=False)
                    nc.tensor.matmul(out=out2[:], lhsT=aggT[:, P:D],
                                     rhs=wc_sb[:, D:2 * D], start=False, stop=True)
                    # relu + LN
                    x_sb = ev_pool.tile([P, D], F32, tag="x")
                    s1 = st_pool.tile([P, 1], F32, tag="s1")
                    nc.scalar.activation(out=x_sb[:], in_=out2[:],
                                         func=mybir.ActivationFunctionType.Relu,
                                         accum_out=s1[:])
                    xsq = ev_pool.tile([P, D], F32, tag="xsq")
                    s2 = st_pool.tile([P, 1], F32, tag="s2")
                    nc.scalar.activation(out=xsq[:], in_=x_sb[:],
                                         func=mybir.ActivationFunctionType.Square,
                                         accum_out=s2[:])
                    mu = st_pool.tile([P, 1], F32, tag="mu")
                    nc.vector.tensor_scalar(out=mu[:], in0=s1[:], scalar1=1.0 / D,
                                            scalar2=None, op0=mybir.AluOpType.mult)
                    musq = st_pool.tile([P, 1], F32, tag="musq")
                    nc.vector.tensor_scalar(out=musq[:], in0=mu[:],
                                            scalar1=mu[:, 0:1], scalar2=LN_EPS,
                                            op0=mybir.AluOpType.mult,
                                            op1=mybir.AluOpType.subtract)
                    var = st_pool.tile([P, 1], F32, tag="var")
                    nc.vector.tensor_scalar(out=var[:], in0=s2[:], scalar1=1.0 / D,
                                            scalar2=musq[:, 0:1],
                                            op0=mybir.AluOpType.mult,
                                            op1=mybir.AluOpType.subtract)
                    sd = st_pool.tile([P, 1], F32, tag="sd")
                    nc.scalar.activation(out=sd[:], in_=var[:],
                                         func=mybir.ActivationFunctionType.Sqrt)
                    rstd = st_pool.tile([P, 1], F32, tag="rstd")
                    nc.vector.reciprocal(out=rstd[:], in_=sd[:])
                    y1 = ev_pool.tile([P, D], F32, tag="y1")
                    nc.vector.tensor_scalar(out=y1[:], in0=x_sb[:],
                                            scalar1=mu[:, 0:1], scalar2=rstd[:, 0:1],
                                            op0=mybir.AluOpType.subtract,
                                            op1=mybir.AluOpType.mult)
                    if apply_affine:
                        y2 = ev_pool.tile([P, D], F32, tag="y2")
                        nc.vector.tensor_tensor(out=y2[:], in0=y1[:], in1=gamma_sb,
                                                op=mybir.AluOpType.mult)
                        y3 = ev_pool.tile([P, D], F32, tag="y3")
                        nc.vector.tensor_tensor(out=y3[:], in0=y2[:], in1=beta_sb,
                                                op=mybir.AluOpType.add)
                        yout = y3
                    else:
                        yout = y1
                    nc.sync.dma_start(out=out_d[w * P:(w + 1) * P, :], in_=yout[:])
    nc.compile()
    return nc


def kernel(feat, W0, W1, W2, a0, a1, a2, ln_gamma, ln_beta,
           src0, dst0, src1, dst1, src2, dst2):
    feat = np.asarray(feat, np.float32)
    prep = _host_prep(feat, np.asarray(W0, np.float32), np.asarray(W1, np.float32),
                      np.asarray(W2, np.float32), a0, a1, a2,
                      [src0, src1, src2], [dst0, dst1, dst2])

    gamma = np.asarray(ln_gamma, np.float32).ravel()
    beta = np.asarray(ln_beta, np.float32).ravel()
    apply_affine = not (np.all(gamma == 1.0) and np.all(beta == 0.0))

    nc = _build_nc(prep["schedule"], prep["total_tiles"], apply_affine)

    tab_bf16 = feat.astype(NP_BF16)
    wc_host = np.zeros((P, 2 * D), np.float32)
    wc_host[:, 0:D] = prep["Wc"][0:P, :]
    wc_host[:, D:2 * D] = prep["Wc"][P:D, :]
    cst_host = np.zeros((P, 2 * P), np.float32)
    cst_host[:, 0:P] = np.arange(P, dtype=np.float32)[None, :]
    cst_host[:, P:2 * P] = np.eye(P, dtype=np.float32)
    gb_host = np.zeros((P, 2 * D), np.float32)
    gb_host[:, 0:D] = gamma[None, :]
    gb_host[:, D:2 * D] = beta[None, :]

    in_maps = []
    for c in range(NC):
        in_maps.append({
            "tab_lo": tab_bf16[:LO_SPLIT],
            "tab_hi": tab_bf16[LO_SPLIT:],
            "idx": prep["idx16"][c],
            "dl": _bf16(prep["dlmat"][c]),
            "wg": _bf16(prep["wgmat"][c]),
            "wc": _bf16(wc_host),
            "cst": _bf16(cst_host),
            "gb": gb_host,
        })

    trace = os.environ.get("BENCH_TRACE", "0") == "1"
    kwargs = {}
    if trace:
        tmpdir = os.environ.get("BENCH_TRACE_DIR", "/tmp/kernel_trace")
        os.makedirs(tmpdir, exist_ok=True)
        kwargs = dict(trace=True, tmpdir=tmpdir)
    res = run_bass_kernel_spmd(nc, in_maps, core_ids=list(range(NC)), **kwargs)
    if trace and res.exec_time_ns:
        print(f"HW exec time: {res.exec_time_ns} ns")

    out = np.concatenate([res.results[c]["out"] for c in range(NC)], axis=0)
    return out[:N].astype(np.float32)
